# revision 23
# baseline (speedup 1.0000x reference)
"""LocalTrittention TRN2 kernel: 8-core batch-data-parallel Bass/Tile implementation.

Problem (B=64, S=256, HID=4096, H=16, D=256, WINDOW=64):
  q,k1,k2,v1,v2 = hs @ W*.T + b*            (5 projections, per-head split)
  s1 = q @ k1^T ; scores = (s1 @ k2^T) * 1/sqrt(D)   (per (b,h), S==D)
  scores[:, S-WINDOW:] = -inf ; probs = softmax(scores)
  out = probs @ (v1+v2)  -> [B,S,HID]

Sharding: batch (64) split across 8 cores (8 batches/core). Weights replicated.
Host prep: layout only (transpose hs shard and the 5 weight matrices so the
contraction index is partition-major); all FLOPs run on device.

Device math runs in fp32r (fp32 rounded to 11 mantissa bits, 4x faster
matmul); accumulation is fp32 in PSUM; softmax in fp32.

Structure (per core, per exec):
  - v1/v2 are only ever used as v1+v2, so the two V GEMMs collapse into one:
    weight tiles wv1+wv2 are summed on DVE and a single GEMM computes v.
  - The output bias (bv1+bv2) is folded into V before the ctx matmul
    (softmax rows sum to 1, so P@(V + 1*bias) == P@V + bias).
  - Token halves (1024 each) are pipelined: projections for half 0, then
    attention for half 0 interleaved with the hst prefetch for half 1, etc.
    Both phases share the single 8-bank PSUM pool.
  - q/k1/k2 PSUM evacuation (+bias) runs on the Activation engine
    (Identity activation with per-partition bias); V evacuation (+bias along
    the free dim) on DVE. This keeps DVE off the PE critical path.
"""

import sys, time

sys.path.insert(0, "/opt/trn_rl_repo")

import numpy as np

import concourse.bass as bass
import concourse.tile as tile
from concourse import bacc, mybir
from concourse.masks import make_identity

B, S, HID = 64, 256, 4096
H, D = 16, 256
WINDOW = 64
SV = S - WINDOW  # valid (unmasked) score columns
SCALE = 1.0 / float(np.sqrt(D))

NCORES = 8
BPC = B // NCORES  # batches per core
T = BPC * S  # tokens per core (2048)
KC = HID // 128  # contraction chunks (32)
HALF = T // 2  # token half (1024)
BPH = BPC // 2  # batches per half (4)

F32 = mybir.dt.float32
F32R = mybir.dt.float32r
AX = mybir.AxisListType.X
EXP = mybir.ActivationFunctionType.Exp
IDENT = mybir.ActivationFunctionType.Identity
COPY = mybir.ActivationFunctionType.Copy


def build_bass(reps=1):
    nc = bacc.Bacc("TRN2", target_bir_lowering=False, debug=True)

    hsT = nc.dram_tensor("hsT", [HID, T], F32, kind="ExternalInput")
    wts = {
        n: nc.dram_tensor(f"w{n}T", [HID, HID], F32, kind="ExternalInput")
        for n in ("q", "k1", "k2", "v1", "v2")
    }
    bqs = {
        n: nc.dram_tensor(f"b{n}", [HID], F32, kind="ExternalInput")
        for n in ("q", "k1", "k2", "v1", "v2")
    }
    outd = nc.dram_tensor("out", [T, HID], F32, kind="ExternalOutput")

    with tile.TileContext(nc) as tc:
        with (
            tc.tile_pool(name="const", bufs=1) as const,
            tc.tile_pool(name="dram", bufs=1, space="DRAM") as dram,
        ):
            # broadcast bias (bv1+bv2) [128, HID] for the ctx epilogue; the two
            # broadcast DMAs ride different queues so they overlap, and tmpb's
            # SBUF is freed (reused by hst) as soon as the add retires
            biasb = const.tile([128, HID], F32)
            with tc.tile_pool(name="btmp", bufs=1) as btmp:
                tmpb = btmp.tile([128, HID], F32)
                nc.sync.dma_start(biasb[:], bqs["v1"].ap().partition_broadcast(128))
                nc.scalar.dma_start(tmpb[:], bqs["v2"].ap().partition_broadcast(128))
                nc.vector.tensor_add(biasb[:], biasb[:], tmpb[:])

            ident = const.tile([128, 128], F32)
            make_identity(nc, ident[:])

            # per-partition bias tiles [128, 32] (o-chunk-major) for q/k1/k2
            bias_sb = {}
            for n in ("q", "k1", "k2"):
                t = const.tile([128, KC], F32, name=f"bias_{n}")
                nc.sync.dma_start(t[:], bqs[n].ap().rearrange("(m p) -> p m", p=128))
                bias_sb[n] = t

            # intermediate DRAM (fp32r): q/k1/k2 stacked [3, HID, T] d-major
            # (one phase-B DMA covers all three), v [T, HID]
            qkTd = dram.tile([3, HID, T], F32R, name="qkTd")
            vd = dram.tile([T, HID], F32R, name="vd")
            proj_idx = {"q": 0, "k1": 1, "k2": 2}

            for _rep in range(reps):
                with (
                    tc.tile_pool(name="hst", bufs=1) as hstp,
                    tc.tile_pool(name="wtile", bufs=7) as wtp,
                    tc.tile_pool(name="evac", bufs=3) as evp,
                    tc.tile_pool(name="psum", bufs=8, space="PSUM") as psump,
                    tc.tile_pool(name="bio", bufs=3) as bio,
                    tc.tile_pool(name="bwork", bufs=3) as bw,
                ):
                    # DMA issue cost scales with bytes; alternate the two
                    # DMA-capable compute queues (Pool/Act) for weight + hst
                    # streams so neither becomes the sole issuer (sync carries
                    # evac writes + phase-B loads). fp32->fp32r is a bitcast
                    # (PE rounds at read), so no gpsimd cast restriction.
                    dma_q = [nc.gpsimd, nc.scalar]

                    def load_hst(hf):
                        cols = slice(hf * HALF, (hf + 1) * HALF)
                        t = hstp.tile([128, KC, HALF], F32R, tag="hst", name="hst")
                        for g in range(8):
                            dma_q[g % 2].dma_start(
                                t[:, g * 4 : (g + 1) * 4, :],
                                hsT.ap()[g * 512 : (g + 1) * 512, cols]
                                .bitcast(F32R)
                                .rearrange("(c p) s -> p c s", p=128),
                            )
                        return t

                    def load_qk(b, h):
                        rows = slice(h * S, (h + 1) * S)
                        colsb = slice(b * S, (b + 1) * S)
                        qk = bio.tile([128, 3, 2, S], F32R, tag="qk", name="qk")
                        for t, q_ in enumerate((nc.sync, nc.gpsimd, nc.scalar)):
                            q_.dma_start(
                                qk[:, t, :, :],
                                qkTd[t, rows, colsb].rearrange(
                                    "(c p) s -> p c s", p=128
                                ),
                            )
                        return qk

                    def load_vt(b, h):
                        rows = slice(h * S, (h + 1) * S)
                        colsb = slice(b * S, (b + 1) * S)
                        vt = bio.tile([128, 2, S], F32R, tag="vt", name="vt", bufs=2)
                        nc.gpsimd.dma_start(
                            vt[:],
                            vd[colsb, rows].rearrange("(c p) s -> p c s", p=128),
                        )
                        return vt

                    def phase_a(hf, hst, prefetch):
                        # q/k1/k2: out-chunk-stationary (W tile), hsT moving
                        for n in ("q", "k1", "k2"):
                            wt = wts[n]
                            for mg in range(8):
                                pss = [
                                    psump.tile([128, 512], F32, tag="ps", name=f"ps{i}")
                                    for i in range(8)
                                ]
                                for k in range(KC):
                                    wtile = wtp.tile(
                                        [128, 512], F32R, tag="wt", name="wtile"
                                    )
                                    dma_q[k % 2].dma_start(
                                        wtile[:],
                                        wt.ap()[
                                            k * 128 : (k + 1) * 128,
                                            mg * 512 : (mg + 1) * 512,
                                        ].bitcast(F32R),
                                    )
                                    for m in range(4):
                                        for nn in range(2):
                                            nc.tensor.matmul(
                                                pss[m * 2 + nn][:],
                                                wtile[:, m * 128 : (m + 1) * 128],
                                                hst[:, k, nn * 512 : (nn + 1) * 512],
                                                start=(k == 0),
                                                stop=(k == KC - 1),
                                            )
                                for m in range(4):
                                    for nn in range(2):
                                        ev = evp.tile(
                                            [128, 512], F32R, tag="ev", name="ev"
                                        )
                                        nc.scalar.activation(
                                            ev[:],
                                            pss[m * 2 + nn][:],
                                            IDENT,
                                            bias=bias_sb[n][:, mg * 4 + m : mg * 4 + m + 1],
                                            scale=1.0,
                                        )
                                        nc.sync.dma_start(
                                            qkTd[
                                                proj_idx[n],
                                                mg * 512 + m * 128 : mg * 512 + (m + 1) * 128,
                                                hf * HALF + nn * 512 : hf * HALF + (nn + 1) * 512,
                                            ],
                                            ev[:],
                                        )

                        # prefetch the first phase-B pairs' q/k1/k2 (ready now)
                        # so attention starts immediately after v completes
                        pre = {}
                        for b, h in prefetch:
                            pre[(b, h)] = load_qk(b, h)

                        # v = hs@((wv1+wv2).T): single GEMM on DVE-summed tiles
                        for ng in range(8):
                            if ng == 2:
                                # vd rows for heads 0/1 (ng 0) landed during ng 1,
                                # so these loads won't block the Pool queue head
                                for b, h in prefetch[:2]:
                                    pre[("vt", b, h)] = load_vt(b, h)
                            pss = [
                                psump.tile([128, 512], F32, tag="ps", name=f"vps{i}")
                                for i in range(8)
                            ]
                            for k in range(KC):
                                wv_tiles = []
                                for wi, n in enumerate(("v1", "v2")):
                                    wtile = wtp.tile(
                                        [128, 512], F32R, tag="wt", name="wvtile"
                                    )
                                    dma_q[wi].dma_start(
                                        wtile[:],
                                        wts[n].ap()[
                                            k * 128 : (k + 1) * 128,
                                            ng * 512 : (ng + 1) * 512,
                                        ].bitcast(F32R),
                                    )
                                    wv_tiles.append(wtile)
                                wsum = wtp.tile([128, 512], F32R, tag="wt", name="wsum")
                                nc.vector.tensor_add(
                                    wsum[:], wv_tiles[0][:], wv_tiles[1][:]
                                )
                                for m in range(8):
                                    nc.tensor.matmul(
                                        pss[m][:],
                                        hst[:, k, m * 128 : (m + 1) * 128],
                                        wsum[:],
                                        start=(k == 0),
                                        stop=(k == KC - 1),
                                    )
                            for m in range(8):
                                ev = evp.tile([128, 512], F32R, tag="ev", name="vev")
                                # Act engine copy: keeps DVE free for wsum adds
                                # (in-order DVE queue would stall the next group)
                                nc.scalar.activation(
                                    ev[:], pss[m][:], COPY, bias=0.0, scale=1.0
                                )
                                nc.sync.dma_start(
                                    vd[
                                        hf * HALF + m * 128 : hf * HALF + (m + 1) * 128,
                                        ng * 512 : (ng + 1) * 512,
                                    ],
                                    ev[:],
                                )
                        return pre

                    def phase_b(hf, pre):
                        for b in range(hf * BPH, (hf + 1) * BPH):
                            for h in range(H):
                                rows = slice(h * S, (h + 1) * S)
                                colsb = slice(b * S, (b + 1) * S)
                                qk = pre.pop((b, h), None)
                                if qk is None:
                                    qk = load_qk(b, h)
                                qt = qk[:, 0, :, :]
                                k1 = qk[:, 1, :, :]
                                k2 = qk[:, 2, :, :]
                                vt = pre.pop(("vt", b, h), None)
                                if vt is None:
                                    vt = load_vt(b, h)

                                # s1T[m,q] = sum_d k1T[d,m] qT[d,q], scaled
                                s1r = bw.tile([128, 2, S], F32R, tag="s1r", name="s1r")
                                for m in range(2):
                                    ps = psump.tile(
                                        [128, 512], F32, tag="ps", name="s1ps"
                                    )
                                    for d_ in range(2):
                                        nc.tensor.matmul(
                                            ps[:, :S],
                                            k1[:, d_, m * 128 : (m + 1) * 128],
                                            qt[:, d_, :],
                                            start=(d_ == 0),
                                            stop=(d_ == 1),
                                        )
                                    nc.vector.tensor_scalar_mul(
                                        s1r[:, m, :], ps[:, :S], SCALE
                                    )

                                # scores[q,j] (full N=S), softmax over j<SV
                                probs = bw.tile(
                                    [128, 2, SV], F32, tag="probs", name="probs", bufs=2
                                )
                                recip = bw.tile([128, 2], F32, tag="recip", name="recip")
                                for q in range(2):
                                    ps = psump.tile(
                                        [128, 512], F32, tag="ps", name="scps"
                                    )
                                    for m in range(2):
                                        nc.tensor.matmul(
                                            ps[:, :S],
                                            s1r[:, m, q * 128 : (q + 1) * 128],
                                            k2[:, m, :],
                                            start=(m == 0),
                                            stop=(m == 1),
                                        )
                                    negmax = bw.tile([128, 1], F32, tag="ngm", name="ngm")
                                    nc.vector.reduce_max(
                                        negmax[:], ps[:, :SV], axis=AX, negate=True
                                    )
                                    sumexp = bw.tile([128, 1], F32, tag="sme", name="sme")
                                    nc.scalar.activation(
                                        probs[:, q, :],
                                        ps[:, :SV],
                                        EXP,
                                        bias=negmax[:],
                                        scale=1.0,
                                        accum_out=sumexp[:],
                                    )
                                    nc.vector.reciprocal(recip[:, q : q + 1], sumexp[:])

                                # transpose probs (valid cols only) -> fp32r
                                ptr = bw.tile([128, 2, S], F32R, tag="ptr", name="ptr")
                                for q in range(2):
                                    pst = psump.tile(
                                        [128, 512], F32, tag="ps", name="pst"
                                    )
                                    nc.tensor.transpose(
                                        pst[:, 0:128], probs[:, q, :128], ident[:]
                                    )
                                    nc.tensor.transpose(
                                        pst[:64, 128:256], probs[:, q, 128:SV], ident[:]
                                    )
                                    # PSUM->SBUF copy on Act: DVE is B's cadence
                                    # limiter, Act has slack
                                    nc.scalar.activation(
                                        ptr[:, q, :], pst[:, :S], COPY, bias=0.0, scale=1.0
                                    )

                                # ctx[q,d] = sum_{j<SV} probsT[j,q] v[j,d];
                                # normalize (DVE) then +bias on gpsimd (idle here)
                                ctxs = bw.tile([128, 2, S], F32, tag="ctxs", name="ctxs", bufs=2)
                                for q in range(2):
                                    ps = psump.tile(
                                        [128, 512], F32, tag="ps", name="ctxps"
                                    )
                                    nc.tensor.matmul(
                                        ps[:, :S],
                                        ptr[:, q, :128],
                                        vt[:, 0, :],
                                        start=True,
                                        stop=False,
                                    )
                                    nc.tensor.matmul(
                                        ps[:, :S],
                                        ptr[:64, q, 128:256],
                                        vt[:64, 1, :],
                                        start=False,
                                        stop=True,
                                    )
                                    nc.vector.tensor_scalar_mul(
                                        ctxs[:, q, :], ps[:, :S], recip[:, q : q + 1]
                                    )
                                    nc.gpsimd.tensor_add(
                                        ctxs[:, q, :], ctxs[:, q, :], biasb[:, rows]
                                    )

                                nc.sync.dma_start(
                                    outd.ap()[colsb, rows].rearrange(
                                        "(c p) s -> p c s", p=128
                                    ),
                                    ctxs[:],
                                )

                    hst = load_hst(0)
                    for hf in range(2):
                        prefetch = [(hf * BPH, h) for h in range(3)]
                        pre = phase_a(hf, hst, prefetch)
                        if hf == 0:
                            hst = load_hst(1)  # prefetch overlaps with phase_b(0)
                        phase_b(hf, pre)

    nc.compile()
    return nc


# ---------------------------------------------------------------------------
# host-side runner (mirrors bass2jax.run_bass_via_pjrt with device-resident
# inputs; weights replicated across cores rather than concatenated)
# ---------------------------------------------------------------------------

_CACHE = {}


def _run(nc, in_maps, n_cores, replicated=(), time_reps=0):
    import jax
    from jax.sharding import Mesh, PartitionSpec, NamedSharding
    from jax.experimental.shard_map import shard_map
    from concourse.bass2jax import (
        install_neuronx_cc_hook,
        _bass_exec_p,
        partition_id_tensor,
    )

    install_neuronx_cc_hook()

    if nc.dbg_addr is not None:
        assert not nc.dbg_callbacks
        in_maps = [
            {**m, nc.dbg_addr.name: np.zeros((1, 2), np.uint32)} for m in in_maps
        ]

    partition_name = nc.partition_id_tensor.name if nc.partition_id_tensor else None

    in_names, out_names, out_avals, zero_outs = [], [], [], []
    for alloc in nc.m.functions[0].allocations:
        if not isinstance(alloc, mybir.MemoryLocationSet):
            continue
        name = alloc.memorylocations[0].name
        if alloc.kind == "ExternalInput":
            if name != partition_name:
                in_names.append(name)
        elif alloc.kind == "ExternalOutput":
            out_names.append(name)
            shape = tuple(alloc.tensor_shape)
            dtype = mybir.dt.np(alloc.dtype)
            out_avals.append(jax.core.ShapedArray(shape, dtype))
            zero_outs.append(np.zeros(shape, dtype))
    n_params = len(in_names)
    n_outs = len(out_avals)
    param_names = list(in_names)
    in_names = in_names + out_names
    if partition_name is not None:
        in_names.append(partition_name)

    donate = tuple(range(n_params, n_params + n_outs))

    def _body(*args):
        operands = list(args)
        if partition_name is not None:
            operands.append(partition_id_tensor())
        outs = _bass_exec_p.bind(
            *operands,
            out_avals=tuple(out_avals),
            in_names=tuple(in_names),
            out_names=tuple(out_names),
            lowering_input_output_aliases=(),
            sim_require_finite=True,
            sim_require_nnan=True,
            nc=nc,
        )
        return tuple(outs)

    devices = jax.devices()[:n_cores]
    mesh = Mesh(np.asarray(devices), ("core",))
    rep = set(replicated)
    in_specs = tuple(
        PartitionSpec() if nm in rep else PartitionSpec("core")
        for nm in param_names
    ) + (PartitionSpec("core"),) * n_outs
    out_specs = (PartitionSpec("core"),) * len(out_names)
    sharded = jax.jit(
        shard_map(
            _body, mesh=mesh, in_specs=in_specs, out_specs=out_specs, check_rep=False
        ),
        donate_argnums=donate,
        keep_unused=True,
    )

    shard_sh = NamedSharding(mesh, PartitionSpec("core"))
    rep_sh = NamedSharding(mesh, PartitionSpec())
    concat_in = []
    for i, nm in enumerate(param_names):
        if nm in rep:
            concat_in.append(jax.device_put(np.asarray(in_maps[0][nm]), rep_sh))
        else:
            concat_in.append(
                jax.device_put(
                    np.concatenate(
                        [np.asarray(in_maps[c][nm]) for c in range(n_cores)], axis=0
                    ),
                    shard_sh,
                )
            )
    jax.block_until_ready(concat_in)

    def fresh_zeros():
        zs = [
            jax.device_put(np.zeros((n_cores * z.shape[0], *z.shape[1:]), z.dtype), shard_sh)
            for z in zero_outs
        ]
        jax.block_until_ready(zs)
        return zs

    t0 = time.perf_counter()
    out_arrs = jax.block_until_ready(sharded(*concat_in, *fresh_zeros()))
    first_call_s = time.perf_counter() - t0
    results = [
        {
            name: np.asarray(out_arrs[i]).reshape(n_cores, *out_avals[i].shape)[c]
            for i, name in enumerate(out_names)
        }
        for c in range(n_cores)
    ]

    # non-donating variant for timing bursts: zeros stay device-resident and
    # are reused across calls (the kernel writes every output element)
    sharded_nd = jax.jit(
        shard_map(
            _body, mesh=mesh, in_specs=in_specs, out_specs=out_specs, check_rep=False
        ),
        keep_unused=True,
    )
    zs_resident = fresh_zeros()

    def timed_burst(m):
        """Enqueue m executions back-to-back, fetch a few bytes of the last
        one's output. Device serializes the execs, so wall ~= dispatch
        overhead + m * exec_time once m*exec exceeds the RPC window."""
        t0 = time.perf_counter()
        outs = None
        for _ in range(m):
            outs = sharded_nd(*concat_in, *zs_resident)
        for o in outs:
            np.asarray(jax.device_get(o.addressable_shards[0].data[0:1, 0:8]))
        return time.perf_counter() - t0

    times = [timed_burst(1) for _ in range(time_reps)]

    return results, times, first_call_s, timed_burst


def kernel(
    hidden_states,
    wq,
    bq,
    wk1,
    bk1,
    wk2,
    bk2,
    wv1,
    bv1,
    wv2,
    bv2,
    _time_reps=0,
    _reps=1,
):
    hs = np.asarray(hidden_states, dtype=np.float32)
    weights = {
        "q": np.asarray(wq, np.float32),
        "k1": np.asarray(wk1, np.float32),
        "k2": np.asarray(wk2, np.float32),
        "v1": np.asarray(wv1, np.float32),
        "v2": np.asarray(wv2, np.float32),
    }
    biases = {
        "q": np.asarray(bq, np.float32),
        "k1": np.asarray(bk1, np.float32),
        "k2": np.asarray(bk2, np.float32),
        "v1": np.asarray(bv1, np.float32),
        "v2": np.asarray(bv2, np.float32),
    }

    if ("nc", _reps) not in _CACHE:
        _CACHE[("nc", _reps)] = build_bass(_reps)
    nc = _CACHE[("nc", _reps)]

    # host prep: layout only (transposes), no arithmetic
    wT = {n: np.ascontiguousarray(w.T) for n, w in weights.items()}
    in_maps = []
    for c in range(NCORES):
        shard = hs[c * BPC : (c + 1) * BPC].reshape(T, HID)
        m = {"hsT": np.ascontiguousarray(shard.T)}
        for n in ("q", "k1", "k2", "v1", "v2"):
            m[f"w{n}T"] = wT[n]
            m[f"b{n}"] = biases[n]
        in_maps.append(m)

    replicated = [f"w{n}T" for n in weights] + [f"b{n}" for n in biases]
    results, times, first_s, burst = _run(
        nc, in_maps, NCORES, replicated=replicated, time_reps=_time_reps
    )
    kernel._last_times = times
    kernel._first_call_s = first_s
    kernel._burst = burst

    out = np.empty((B, S, HID), np.float32)
    for c in range(NCORES):
        out[c * BPC : (c + 1) * BPC] = results[c]["out"].reshape(BPC, S, HID)
    return out


# revision 30
# speedup vs baseline: 1.0217x; 1.0217x over previous
"""LocalTrittention TRN2 kernel: 8-core batch-data-parallel Bass/Tile implementation.

Problem (B=64, S=256, HID=4096, H=16, D=256, WINDOW=64):
  q,k1,k2,v1,v2 = hs @ W*.T + b*            (5 projections, per-head split)
  s1 = q @ k1^T ; scores = (s1 @ k2^T) * 1/sqrt(D)   (per (b,h), S==D)
  scores[:, S-WINDOW:] = -inf ; probs = softmax(scores)
  out = probs @ (v1+v2)  -> [B,S,HID]

Sharding: batch (64) split across 8 cores (8 batches/core). Weights replicated.
Host prep: layout only (transpose hs shard and the 5 weight matrices so the
contraction index is partition-major); all FLOPs run on device.

Device math runs in fp32r (fp32 rounded to 11 mantissa bits, 4x faster
matmul); accumulation is fp32 in PSUM; softmax in fp32.

Structure (per core, per exec):
  - v1/v2 are only ever used as v1+v2, so the two V GEMMs collapse into one:
    weight tiles wv1+wv2 are summed on DVE and a single GEMM computes v.
  - The output bias (bv1+bv2) is folded into V before the ctx matmul
    (softmax rows sum to 1, so P@(V + 1*bias) == P@V + bias).
  - Token halves (1024 each) are pipelined: projections for half 0, then
    attention for half 0 interleaved with the hst prefetch for half 1, etc.
    Both phases share the single 8-bank PSUM pool.
  - q/k1/k2 PSUM evacuation (+bias) runs on the Activation engine
    (Identity activation with per-partition bias); V evacuation (+bias along
    the free dim) on DVE. This keeps DVE off the PE critical path.
"""

import sys, time

sys.path.insert(0, "/opt/trn_rl_repo")

import numpy as np

import concourse.bass as bass
import concourse.tile as tile
from concourse import bacc, mybir
from concourse.masks import make_identity

B, S, HID = 64, 256, 4096
H, D = 16, 256
WINDOW = 64
SV = S - WINDOW  # valid (unmasked) score columns
SCALE = 1.0 / float(np.sqrt(D))

NCORES = 8
BPC = B // NCORES  # batches per core
T = BPC * S  # tokens per core (2048)
KC = HID // 128  # contraction chunks (32)
HALF = T // 2  # token half (1024)
BPH = BPC // 2  # batches per half (4)
PHALF = BPH * SV  # packed k2/v tokens per half (768)
PT = 2 * PHALF  # packed k2/v tokens per core (1536)

F32 = mybir.dt.float32
F32R = mybir.dt.float32r
AX = mybir.AxisListType.X
EXP = mybir.ActivationFunctionType.Exp
IDENT = mybir.ActivationFunctionType.Identity
COPY = mybir.ActivationFunctionType.Copy


def build_bass(reps=1):
    nc = bacc.Bacc("TRN2", target_bir_lowering=False, debug=True)

    hsT = nc.dram_tensor("hsT", [HID, T], F32, kind="ExternalInput")
    wts = {
        n: nc.dram_tensor(f"w{n}T", [HID, HID], F32, kind="ExternalInput")
        for n in ("q", "k1", "k2", "v1", "v2")
    }
    bqs = {
        n: nc.dram_tensor(f"b{n}", [HID], F32, kind="ExternalInput")
        for n in ("q", "k1", "k2", "v1", "v2")
    }
    outd = nc.dram_tensor("out", [T, HID], F32, kind="ExternalOutput")

    with tile.TileContext(nc) as tc:
        with (
            tc.tile_pool(name="const", bufs=1) as const,
            tc.tile_pool(name="dram", bufs=1, space="DRAM") as dram,
        ):
            # broadcast bias (bv1+bv2) [128, HID] for the ctx epilogue; the two
            # broadcast DMAs ride different queues so they overlap, and tmpb's
            # SBUF is freed (reused by hst) as soon as the add retires
            biasb = const.tile([128, HID], F32)
            with tc.tile_pool(name="btmp", bufs=1) as btmp:
                tmpb = btmp.tile([128, HID], F32)
                nc.sync.dma_start(biasb[:], bqs["v1"].ap().partition_broadcast(128))
                nc.scalar.dma_start(tmpb[:], bqs["v2"].ap().partition_broadcast(128))
                nc.vector.tensor_add(biasb[:], biasb[:], tmpb[:])

            ident = const.tile([128, 128], F32)
            make_identity(nc, ident[:])

            # per-partition bias tiles [128, 32] (o-chunk-major) for q/k1/k2
            bias_sb = {}
            for n in ("q", "k1", "k2"):
                t = const.tile([128, KC], F32, name=f"bias_{n}")
                nc.sync.dma_start(t[:], bqs[n].ap().rearrange("(m p) -> p m", p=128))
                bias_sb[n] = t

            # intermediate DRAM (fp32r): q/k1 stacked [2, HID, T] d-major.
            # The local mask kills key tokens >= SV per batch, so k2 and v are
            # only ever needed for the first SV=192 tokens of each batch: both
            # use packed layouts (PT = 8 batches * 192 tokens) and their GEMMs
            # skip 25% of the work.
            qk12Td = dram.tile([2, HID, T], F32R, name="qk12Td")
            k2Td = dram.tile([HID, PT], F32R, name="k2Td")
            vdc = dram.tile([PT, HID], F32R, name="vdc")
            proj_idx = {"q": 0, "k1": 1}

            for _rep in range(reps):
                with (
                    tc.tile_pool(name="hst", bufs=1) as hstp,
                    tc.tile_pool(name="wtile", bufs=7) as wtp,
                    tc.tile_pool(name="evac", bufs=3) as evp,
                    tc.tile_pool(name="psum", bufs=8, space="PSUM") as psump,
                    tc.tile_pool(name="bio", bufs=3) as bio,
                    tc.tile_pool(name="bwork", bufs=3) as bw,
                ):
                    # DMA issue cost scales with bytes; alternate the two
                    # DMA-capable compute queues (Pool/Act) for weight + hst
                    # streams so neither becomes the sole issuer (sync carries
                    # evac writes + phase-B loads). fp32->fp32r is a bitcast
                    # (PE rounds at read), so no gpsimd cast restriction.
                    dma_q = [nc.gpsimd, nc.scalar]

                    def load_hst(hf):
                        cols = slice(hf * HALF, (hf + 1) * HALF)
                        t = hstp.tile([128, KC, HALF], F32R, tag="hst", name="hst")
                        for g in range(8):
                            dma_q[g % 2].dma_start(
                                t[:, g * 4 : (g + 1) * 4, :],
                                hsT.ap()[g * 512 : (g + 1) * 512, cols]
                                .bitcast(F32R)
                                .rearrange("(c p) s -> p c s", p=128),
                            )
                        return t

                    def load_qk(b, h):
                        rows = slice(h * S, (h + 1) * S)
                        hf, bb = divmod(b, BPH)
                        cv = hf * HALF + bb * SV  # valid tokens (192)
                        cm = hf * HALF + PHALF + bb * WINDOW  # masked (64)
                        qk = bio.tile([128, 2, 2, S], F32R, tag="qk", name="qk")
                        for t, q_ in enumerate((nc.sync, nc.gpsimd)):
                            q_.dma_start(
                                qk[:, t, :, :SV],
                                qk12Td[t, rows, cv : cv + SV].rearrange(
                                    "(c p) s -> p c s", p=128
                                ),
                            )
                            q_.dma_start(
                                qk[:, t, :, SV:],
                                qk12Td[t, rows, cm : cm + WINDOW].rearrange(
                                    "(c p) s -> p c s", p=128
                                ),
                            )
                        return qk

                    def load_k2(b, h):
                        rows = slice(h * S, (h + 1) * S)
                        k2 = bio.tile([128, 2, SV], F32R, tag="k2", name="k2")
                        nc.sync.dma_start(
                            k2[:],
                            k2Td[rows, b * SV : (b + 1) * SV].rearrange(
                                "(c p) s -> p c s", p=128
                            ),
                        )
                        return k2

                    def load_vt(b, h):
                        rows = slice(h * S, (h + 1) * S)
                        vt = bio.tile([128, 2, S], F32R, tag="vt", name="vt", bufs=2)
                        nc.gpsimd.dma_start(
                            vt[:, 0, :], vdc[b * SV : b * SV + 128, rows]
                        )
                        nc.gpsimd.dma_start(
                            vt[:64, 1, :], vdc[b * SV + 128 : (b + 1) * SV, rows]
                        )
                        return vt

                    # The host permutes each half's token columns to
                    # [768 valid (4 batches x 192)] [256 masked (4 x 64)], so
                    # the k2/v GEMMs read contiguous packed slices and q/k1
                    # GEMM columns are simply the permuted token space.

                    def phase_a(hf, hst, prefetch):
                        # q/k1: out-chunk-stationary (W tile), hsT moving
                        for n in ("q", "k1"):
                            wt = wts[n]
                            for mg in range(8):
                                pss = [
                                    psump.tile([128, 512], F32, tag="ps", name=f"ps{i}")
                                    for i in range(8)
                                ]
                                for k in range(KC):
                                    wtile = wtp.tile(
                                        [128, 512], F32R, tag="wt", name="wtile"
                                    )
                                    dma_q[k % 2].dma_start(
                                        wtile[:],
                                        wt.ap()[
                                            k * 128 : (k + 1) * 128,
                                            mg * 512 : (mg + 1) * 512,
                                        ].bitcast(F32R),
                                    )
                                    for m in range(4):
                                        for nn in range(2):
                                            nc.tensor.matmul(
                                                pss[m * 2 + nn][:],
                                                wtile[:, m * 128 : (m + 1) * 128],
                                                hst[:, k, nn * 512 : (nn + 1) * 512],
                                                start=(k == 0),
                                                stop=(k == KC - 1),
                                            )
                                for m in range(4):
                                    for nn in range(2):
                                        ev = evp.tile(
                                            [128, 512], F32R, tag="ev", name="ev"
                                        )
                                        nc.scalar.activation(
                                            ev[:],
                                            pss[m * 2 + nn][:],
                                            IDENT,
                                            bias=bias_sb[n][:, mg * 4 + m : mg * 4 + m + 1],
                                            scale=1.0,
                                        )
                                        nc.sync.dma_start(
                                            qk12Td[
                                                proj_idx[n],
                                                mg * 512 + m * 128 : mg * 512 + (m + 1) * 128,
                                                hf * HALF + nn * 512 : hf * HALF + (nn + 1) * 512,
                                            ],
                                            ev[:],
                                        )

                        # k2: same structure but packed tokens - each nn group
                        # covers 2 batches x 192 valid tokens = 384 columns
                        for mg in range(8):
                            pss = [
                                psump.tile([128, 512], F32, tag="ps", name=f"kps{i}")
                                for i in range(8)
                            ]
                            for k in range(KC):
                                wtile = wtp.tile(
                                    [128, 512], F32R, tag="wt", name="wtile"
                                )
                                dma_q[k % 2].dma_start(
                                    wtile[:],
                                    wts["k2"].ap()[
                                        k * 128 : (k + 1) * 128,
                                        mg * 512 : (mg + 1) * 512,
                                    ].bitcast(F32R),
                                )
                                for m in range(4):
                                    for nn in range(2):
                                        nc.tensor.matmul(
                                            pss[m * 2 + nn][:, :2 * SV],
                                            wtile[:, m * 128 : (m + 1) * 128],
                                            hst[:, k, nn * 2 * SV : (nn + 1) * 2 * SV],
                                            start=(k == 0),
                                            stop=(k == KC - 1),
                                        )
                            for m in range(4):
                                for nn in range(2):
                                    ev = evp.tile(
                                        [128, 512], F32R, tag="ev", name="kev"
                                    )
                                    nc.scalar.activation(
                                        ev[:, :2 * SV],
                                        pss[m * 2 + nn][:, :2 * SV],
                                        IDENT,
                                        bias=bias_sb["k2"][:, mg * 4 + m : mg * 4 + m + 1],
                                        scale=1.0,
                                    )
                                    nc.sync.dma_start(
                                        k2Td[
                                            mg * 512 + m * 128 : mg * 512 + (m + 1) * 128,
                                            hf * PHALF + nn * 2 * SV : hf * PHALF + (nn + 1) * 2 * SV,
                                        ],
                                        ev[:, :2 * SV],
                                    )

                        # prefetch the first phase-B pairs' q/k1/k2 (ready now)
                        # so attention starts immediately after v completes
                        pre = {}
                        for b, h in prefetch:
                            pre[(b, h)] = load_qk(b, h)
                            pre[("k2", b, h)] = load_k2(b, h)

                        # v = hs@((wv1+wv2).T): single GEMM on DVE-summed tiles
                        for ng in range(8):
                            if ng == 2:
                                # vd rows for heads 0/1 (ng 0) landed during ng 1,
                                # so these loads won't block the Pool queue head
                                for b, h in prefetch[:2]:
                                    pre[("vt", b, h)] = load_vt(b, h)
                            pss = [
                                psump.tile([128, 512], F32, tag="ps", name=f"vps{i}")
                                for i in range(6)
                            ]
                            for k in range(KC):
                                wv_tiles = []
                                for wi, n in enumerate(("v1", "v2")):
                                    wtile = wtp.tile(
                                        [128, 512], F32R, tag="wt", name="wvtile"
                                    )
                                    dma_q[wi].dma_start(
                                        wtile[:],
                                        wts[n].ap()[
                                            k * 128 : (k + 1) * 128,
                                            ng * 512 : (ng + 1) * 512,
                                        ].bitcast(F32R),
                                    )
                                    wv_tiles.append(wtile)
                                wsum = wtp.tile([128, 512], F32R, tag="wt", name="wsum")
                                nc.vector.tensor_add(
                                    wsum[:], wv_tiles[0][:], wv_tiles[1][:]
                                )
                                for m in range(6):
                                    nc.tensor.matmul(
                                        pss[m][:],
                                        hst[:, k, m * 128 : (m + 1) * 128],
                                        wsum[:],
                                        start=(k == 0),
                                        stop=(k == KC - 1),
                                    )
                            for m in range(6):
                                ev = evp.tile([128, 512], F32R, tag="ev", name="vev")
                                # Act engine copy: keeps DVE free for wsum adds
                                # (in-order DVE queue would stall the next group)
                                nc.scalar.activation(
                                    ev[:], pss[m][:], COPY, bias=0.0, scale=1.0
                                )
                                nc.sync.dma_start(
                                    vdc[
                                        hf * PHALF + m * 128 : hf * PHALF + (m + 1) * 128,
                                        ng * 512 : (ng + 1) * 512,
                                    ],
                                    ev[:],
                                )
                        return pre

                    def phase_b(hf, pre):
                        for b in range(hf * BPH, (hf + 1) * BPH):
                            for h in range(H):
                                rows = slice(h * S, (h + 1) * S)
                                colsb = slice(b * S, (b + 1) * S)
                                qk = pre.pop((b, h), None)
                                if qk is None:
                                    qk = load_qk(b, h)
                                qt = qk[:, 0, :, :]
                                k1 = qk[:, 1, :, :]
                                k2 = pre.pop(("k2", b, h), None)
                                if k2 is None:
                                    k2 = load_k2(b, h)
                                vt = pre.pop(("vt", b, h), None)
                                if vt is None:
                                    vt = load_vt(b, h)

                                # s1T[m,q] = sum_d k1T[d,m] qT[d,q], scaled
                                s1r = bw.tile([128, 2, S], F32R, tag="s1r", name="s1r")
                                for m in range(2):
                                    ps = psump.tile(
                                        [128, 512], F32, tag="ps", name="s1ps"
                                    )
                                    for d_ in range(2):
                                        nc.tensor.matmul(
                                            ps[:, :S],
                                            k1[:, d_, m * 128 : (m + 1) * 128],
                                            qt[:, d_, :],
                                            start=(d_ == 0),
                                            stop=(d_ == 1),
                                        )
                                    nc.vector.tensor_scalar_mul(
                                        s1r[:, m, :], ps[:, :S], SCALE
                                    )

                                # scores[q,j] (full N=S), softmax over j<SV
                                probs = bw.tile(
                                    [128, 2, SV], F32, tag="probs", name="probs", bufs=2
                                )
                                recip = bw.tile([128, 2], F32, tag="recip", name="recip")
                                for q in range(2):
                                    ps = psump.tile(
                                        [128, 512], F32, tag="ps", name="scps"
                                    )
                                    for m in range(2):
                                        nc.tensor.matmul(
                                            ps[:, :SV],
                                            s1r[:, m, q * 128 : (q + 1) * 128],
                                            k2[:, m, :],
                                            start=(m == 0),
                                            stop=(m == 1),
                                        )
                                    negmax = bw.tile([128, 1], F32, tag="ngm", name="ngm")
                                    nc.vector.reduce_max(
                                        negmax[:], ps[:, :SV], axis=AX, negate=True
                                    )
                                    sumexp = bw.tile([128, 1], F32, tag="sme", name="sme")
                                    nc.scalar.activation(
                                        probs[:, q, :],
                                        ps[:, :SV],
                                        EXP,
                                        bias=negmax[:],
                                        scale=1.0,
                                        accum_out=sumexp[:],
                                    )
                                    nc.vector.reciprocal(recip[:, q : q + 1], sumexp[:])

                                # transpose probs (valid cols only) -> fp32r
                                ptr = bw.tile([128, 2, S], F32R, tag="ptr", name="ptr")
                                for q in range(2):
                                    pst = psump.tile(
                                        [128, 512], F32, tag="ps", name="pst"
                                    )
                                    nc.tensor.transpose(
                                        pst[:, 0:128], probs[:, q, :128], ident[:]
                                    )
                                    nc.tensor.transpose(
                                        pst[:64, 128:256], probs[:, q, 128:SV], ident[:]
                                    )
                                    # PSUM->SBUF copy on Act: DVE is B's cadence
                                    # limiter, Act has slack
                                    nc.scalar.activation(
                                        ptr[:, q, :], pst[:, :S], COPY, bias=0.0, scale=1.0
                                    )

                                # ctx[q,d] = sum_{j<SV} probsT[j,q] v[j,d];
                                # normalize (DVE) then +bias on gpsimd (idle here)
                                ctxs = bw.tile([128, 2, S], F32, tag="ctxs", name="ctxs", bufs=2)
                                for q in range(2):
                                    ps = psump.tile(
                                        [128, 512], F32, tag="ps", name="ctxps"
                                    )
                                    nc.tensor.matmul(
                                        ps[:, :S],
                                        ptr[:, q, :128],
                                        vt[:, 0, :],
                                        start=True,
                                        stop=False,
                                    )
                                    nc.tensor.matmul(
                                        ps[:, :S],
                                        ptr[:64, q, 128:256],
                                        vt[:64, 1, :],
                                        start=False,
                                        stop=True,
                                    )
                                    nc.vector.tensor_scalar_mul(
                                        ctxs[:, q, :], ps[:, :S], recip[:, q : q + 1]
                                    )
                                    nc.gpsimd.tensor_add(
                                        ctxs[:, q, :], ctxs[:, q, :], biasb[:, rows]
                                    )

                                nc.sync.dma_start(
                                    outd.ap()[colsb, rows].rearrange(
                                        "(c p) s -> p c s", p=128
                                    ),
                                    ctxs[:],
                                )

                    hst = load_hst(0)
                    for hf in range(2):
                        prefetch = [(hf * BPH, h) for h in range(3)]
                        pre = phase_a(hf, hst, prefetch)
                        if hf == 0:
                            hst = load_hst(1)  # prefetch overlaps with phase_b(0)
                        phase_b(hf, pre)

    nc.compile()
    return nc


# ---------------------------------------------------------------------------
# host-side runner (mirrors bass2jax.run_bass_via_pjrt with device-resident
# inputs; weights replicated across cores rather than concatenated)
# ---------------------------------------------------------------------------

_CACHE = {}


def _run(nc, in_maps, n_cores, replicated=(), time_reps=0):
    import jax
    from jax.sharding import Mesh, PartitionSpec, NamedSharding
    from jax.experimental.shard_map import shard_map
    from concourse.bass2jax import (
        install_neuronx_cc_hook,
        _bass_exec_p,
        partition_id_tensor,
    )

    install_neuronx_cc_hook()

    if nc.dbg_addr is not None:
        assert not nc.dbg_callbacks
        in_maps = [
            {**m, nc.dbg_addr.name: np.zeros((1, 2), np.uint32)} for m in in_maps
        ]

    partition_name = nc.partition_id_tensor.name if nc.partition_id_tensor else None

    in_names, out_names, out_avals, zero_outs = [], [], [], []
    for alloc in nc.m.functions[0].allocations:
        if not isinstance(alloc, mybir.MemoryLocationSet):
            continue
        name = alloc.memorylocations[0].name
        if alloc.kind == "ExternalInput":
            if name != partition_name:
                in_names.append(name)
        elif alloc.kind == "ExternalOutput":
            out_names.append(name)
            shape = tuple(alloc.tensor_shape)
            dtype = mybir.dt.np(alloc.dtype)
            out_avals.append(jax.core.ShapedArray(shape, dtype))
            zero_outs.append(np.zeros(shape, dtype))
    n_params = len(in_names)
    n_outs = len(out_avals)
    param_names = list(in_names)
    in_names = in_names + out_names
    if partition_name is not None:
        in_names.append(partition_name)

    donate = tuple(range(n_params, n_params + n_outs))

    def _body(*args):
        operands = list(args)
        if partition_name is not None:
            operands.append(partition_id_tensor())
        outs = _bass_exec_p.bind(
            *operands,
            out_avals=tuple(out_avals),
            in_names=tuple(in_names),
            out_names=tuple(out_names),
            lowering_input_output_aliases=(),
            sim_require_finite=True,
            sim_require_nnan=True,
            nc=nc,
        )
        return tuple(outs)

    devices = jax.devices()[:n_cores]
    mesh = Mesh(np.asarray(devices), ("core",))
    rep = set(replicated)
    in_specs = tuple(
        PartitionSpec() if nm in rep else PartitionSpec("core")
        for nm in param_names
    ) + (PartitionSpec("core"),) * n_outs
    out_specs = (PartitionSpec("core"),) * len(out_names)
    sharded = jax.jit(
        shard_map(
            _body, mesh=mesh, in_specs=in_specs, out_specs=out_specs, check_rep=False
        ),
        donate_argnums=donate,
        keep_unused=True,
    )

    shard_sh = NamedSharding(mesh, PartitionSpec("core"))
    rep_sh = NamedSharding(mesh, PartitionSpec())
    concat_in = []
    for i, nm in enumerate(param_names):
        if nm in rep:
            concat_in.append(jax.device_put(np.asarray(in_maps[0][nm]), rep_sh))
        else:
            concat_in.append(
                jax.device_put(
                    np.concatenate(
                        [np.asarray(in_maps[c][nm]) for c in range(n_cores)], axis=0
                    ),
                    shard_sh,
                )
            )
    jax.block_until_ready(concat_in)

    def fresh_zeros():
        zs = [
            jax.device_put(np.zeros((n_cores * z.shape[0], *z.shape[1:]), z.dtype), shard_sh)
            for z in zero_outs
        ]
        jax.block_until_ready(zs)
        return zs

    t0 = time.perf_counter()
    out_arrs = jax.block_until_ready(sharded(*concat_in, *fresh_zeros()))
    first_call_s = time.perf_counter() - t0
    results = [
        {
            name: np.asarray(out_arrs[i]).reshape(n_cores, *out_avals[i].shape)[c]
            for i, name in enumerate(out_names)
        }
        for c in range(n_cores)
    ]

    # non-donating variant for timing bursts: zeros stay device-resident and
    # are reused across calls (the kernel writes every output element)
    sharded_nd = jax.jit(
        shard_map(
            _body, mesh=mesh, in_specs=in_specs, out_specs=out_specs, check_rep=False
        ),
        keep_unused=True,
    )
    zs_resident = fresh_zeros()

    def timed_burst(m):
        """Enqueue m executions back-to-back, fetch a few bytes of the last
        one's output. Device serializes the execs, so wall ~= dispatch
        overhead + m * exec_time once m*exec exceeds the RPC window."""
        t0 = time.perf_counter()
        outs = None
        for _ in range(m):
            outs = sharded_nd(*concat_in, *zs_resident)
        for o in outs:
            np.asarray(jax.device_get(o.addressable_shards[0].data[0:1, 0:8]))
        return time.perf_counter() - t0

    times = [timed_burst(1) for _ in range(time_reps)]

    return results, times, first_call_s, timed_burst


def kernel(
    hidden_states,
    wq,
    bq,
    wk1,
    bk1,
    wk2,
    bk2,
    wv1,
    bv1,
    wv2,
    bv2,
    _time_reps=0,
    _reps=1,
):
    hs = np.asarray(hidden_states, dtype=np.float32)
    weights = {
        "q": np.asarray(wq, np.float32),
        "k1": np.asarray(wk1, np.float32),
        "k2": np.asarray(wk2, np.float32),
        "v1": np.asarray(wv1, np.float32),
        "v2": np.asarray(wv2, np.float32),
    }
    biases = {
        "q": np.asarray(bq, np.float32),
        "k1": np.asarray(bk1, np.float32),
        "k2": np.asarray(bk2, np.float32),
        "v1": np.asarray(bv1, np.float32),
        "v2": np.asarray(bv2, np.float32),
    }

    if ("nc", _reps) not in _CACHE:
        _CACHE[("nc", _reps)] = build_bass(_reps)
    nc = _CACHE[("nc", _reps)]

    # host prep: layout only (transpose + token-column permutation), no
    # arithmetic. Each half's tokens are reordered to [4 batches x 192
    # mask-valid][4 x 64 masked] so the k2/v GEMMs read contiguous packed
    # slices on device.
    wT = {n: np.ascontiguousarray(w.T) for n, w in weights.items()}
    perm = np.concatenate(
        [
            np.concatenate(
                [
                    np.arange(hf * HALF + bb * S, hf * HALF + bb * S + SV)
                    for bb in range(BPH)
                ]
                + [
                    np.arange(hf * HALF + bb * S + SV, hf * HALF + (bb + 1) * S)
                    for bb in range(BPH)
                ]
            )
            for hf in range(2)
        ]
    )
    in_maps = []
    for c in range(NCORES):
        shard = hs[c * BPC : (c + 1) * BPC].reshape(T, HID)[perm]
        m = {"hsT": np.ascontiguousarray(shard.T)}
        for n in ("q", "k1", "k2", "v1", "v2"):
            m[f"w{n}T"] = wT[n]
            m[f"b{n}"] = biases[n]
        in_maps.append(m)

    replicated = [f"w{n}T" for n in weights] + [f"b{n}" for n in biases]
    results, times, first_s, burst = _run(
        nc, in_maps, NCORES, replicated=replicated, time_reps=_time_reps
    )
    kernel._last_times = times
    kernel._first_call_s = first_s
    kernel._burst = burst

    out = np.empty((B, S, HID), np.float32)
    for c in range(NCORES):
        out[c * BPC : (c + 1) * BPC] = results[c]["out"].reshape(BPC, S, HID)
    return out


# revision 34
# speedup vs baseline: 1.0536x; 1.0312x over previous
"""LocalTrittention TRN2 kernel: 8-core batch-data-parallel Bass/Tile implementation.

Problem (B=64, S=256, HID=4096, H=16, D=256, WINDOW=64):
  q,k1,k2,v1,v2 = hs @ W*.T + b*            (5 projections, per-head split)
  s1 = q @ k1^T ; scores = (s1 @ k2^T) * 1/sqrt(D)   (per (b,h), S==D)
  scores[:, S-WINDOW:] = -inf ; probs = softmax(scores)
  out = probs @ (v1+v2)  -> [B,S,HID]

Sharding: batch (64) split across 8 cores (8 batches/core). Weights replicated.
Host prep: layout only (transpose hs shard and the 5 weight matrices so the
contraction index is partition-major); all FLOPs run on device.

Device math runs in fp32r (fp32 rounded to 11 mantissa bits, 4x faster
matmul); accumulation is fp32 in PSUM; softmax in fp32.

Structure (per core, per exec):
  - v1/v2 are only ever used as v1+v2, so the two V GEMMs collapse into one:
    weight tiles wv1+wv2 are summed on DVE and a single GEMM computes v.
  - The output bias (bv1+bv2) is folded into V before the ctx matmul
    (softmax rows sum to 1, so P@(V + 1*bias) == P@V + bias).
  - Token halves (1024 each) are pipelined: projections for half 0, then
    attention for half 0 interleaved with the hst prefetch for half 1, etc.
    Both phases share the single 8-bank PSUM pool.
  - q/k1/k2 PSUM evacuation (+bias) runs on the Activation engine
    (Identity activation with per-partition bias); V evacuation (+bias along
    the free dim) on DVE. This keeps DVE off the PE critical path.
"""

import sys, time

sys.path.insert(0, "/opt/trn_rl_repo")

import numpy as np

import concourse.bass as bass
import concourse.tile as tile
from concourse import bacc, mybir
from concourse.masks import make_identity

B, S, HID = 64, 256, 4096
H, D = 16, 256
WINDOW = 64
SV = S - WINDOW  # valid (unmasked) score columns
SCALE = 1.0 / float(np.sqrt(D))

NCORES = 8
BPC = B // NCORES  # batches per core
T = BPC * S  # tokens per core (2048)
KC = HID // 128  # contraction chunks (32)
HALF = T // 2  # token half (1024)
BPH = BPC // 2  # batches per half (4)
PHALF = BPH * SV  # packed k2/v tokens per half (768)
PT = 2 * PHALF  # packed k2/v tokens per core (1536)

F32 = mybir.dt.float32
F32R = mybir.dt.float32r
AX = mybir.AxisListType.X
EXP = mybir.ActivationFunctionType.Exp
IDENT = mybir.ActivationFunctionType.Identity
COPY = mybir.ActivationFunctionType.Copy


def build_bass(reps=1):
    nc = bacc.Bacc("TRN2", target_bir_lowering=False, debug=True)

    hsT = nc.dram_tensor("hsT", [HID, T], F32, kind="ExternalInput")
    wts = {
        n: nc.dram_tensor(f"w{n}T", [HID, HID], F32, kind="ExternalInput")
        for n in ("q", "k1", "k2", "v1", "v2")
    }
    bqs = {
        n: nc.dram_tensor(f"b{n}", [HID], F32, kind="ExternalInput")
        for n in ("q", "k1", "k2", "v1", "v2")
    }
    outd = nc.dram_tensor("out", [T, HID], F32, kind="ExternalOutput")

    with tile.TileContext(nc) as tc:
        with (
            tc.tile_pool(name="const", bufs=1) as const,
            tc.tile_pool(name="dram", bufs=1, space="DRAM") as dram,
        ):
            # broadcast bias (bv1+bv2) [128, HID] for the ctx epilogue; the two
            # broadcast DMAs ride different queues so they overlap, and tmpb's
            # SBUF is freed (reused by hst) as soon as the add retires
            biasb = const.tile([128, HID], F32)
            with tc.tile_pool(name="btmp", bufs=1) as btmp:
                tmpb = btmp.tile([128, HID], F32)
                nc.sync.dma_start(biasb[:], bqs["v1"].ap().partition_broadcast(128))
                nc.scalar.dma_start(tmpb[:], bqs["v2"].ap().partition_broadcast(128))
                nc.vector.tensor_add(biasb[:], biasb[:], tmpb[:])

            ident = const.tile([128, 128], F32)
            make_identity(nc, ident[:])

            # per-partition bias tiles [128, 32] (o-chunk-major) for q/k1/k2
            bias_sb = {}
            for n in ("q", "k1", "k2"):
                t = const.tile([128, KC], F32, name=f"bias_{n}")
                nc.sync.dma_start(t[:], bqs[n].ap().rearrange("(m p) -> p m", p=128))
                bias_sb[n] = t

            # intermediate DRAM (fp32r): q/k1 stacked [2, HID, T] d-major.
            # The local mask kills key tokens >= SV per batch, so k2 and v are
            # only ever needed for the first SV=192 tokens of each batch: both
            # use packed layouts (PT = 8 batches * 192 tokens) and their GEMMs
            # skip 25% of the work.
            qk12Td = dram.tile([2, HID, T], F32R, name="qk12Td")
            k2Td = dram.tile([HID, PT], F32R, name="k2Td")
            vdc = dram.tile([PT, HID], F32R, name="vdc")
            proj_idx = {"q": 0, "k1": 1}

            for _rep in range(reps):
                with (
                    tc.tile_pool(name="hst", bufs=1) as hstp,
                    tc.tile_pool(name="wtile", bufs=4) as wtp,
                    tc.tile_pool(name="evac", bufs=2) as evp,
                    tc.tile_pool(name="psum", bufs=8, space="PSUM") as psump,
                    tc.tile_pool(name="bio", bufs=3) as bio,
                    tc.tile_pool(name="bwork", bufs=3) as bw,
                ):
                    # DMA issue cost scales with bytes; alternate the two
                    # DMA-capable compute queues (Pool/Act) for weight + hst
                    # streams so neither becomes the sole issuer (sync carries
                    # evac writes + phase-B loads). fp32->fp32r is a bitcast
                    # (PE rounds at read), so no gpsimd cast restriction.
                    dma_q = [nc.gpsimd, nc.scalar]

                    def load_hst(hf):
                        cols = slice(hf * HALF, (hf + 1) * HALF)
                        t = hstp.tile([128, KC, HALF], F32R, tag="hst", name="hst")
                        for g in range(8):
                            dma_q[g % 2].dma_start(
                                t[:, g * 4 : (g + 1) * 4, :],
                                hsT.ap()[g * 512 : (g + 1) * 512, cols]
                                .bitcast(F32R)
                                .rearrange("(c p) s -> p c s", p=128),
                            )
                        return t

                    def load_qk(b, h):
                        rows = slice(h * S, (h + 1) * S)
                        colsb = slice(b * S, (b + 1) * S)
                        qk = bio.tile([128, 2, 2, S], F32R, tag="qk", name="qk", bufs=2)
                        for t, q_ in enumerate((nc.sync, nc.gpsimd)):
                            q_.dma_start(
                                qk[:, t, :, :],
                                qk12Td[t, rows, colsb].rearrange(
                                    "(c p) s -> p c s", p=128
                                ),
                            )
                        return qk

                    def load_k2(b, h):
                        rows = slice(h * S, (h + 1) * S)
                        k2 = bio.tile([128, 2, SV], F32R, tag="k2", name="k2")
                        nc.sync.dma_start(
                            k2[:],
                            k2Td[rows, b * SV : (b + 1) * SV].rearrange(
                                "(c p) s -> p c s", p=128
                            ),
                        )
                        return k2

                    def load_vt(b, h):
                        rows = slice(h * S, (h + 1) * S)
                        vt = bio.tile([128, 2, S], F32R, tag="vt", name="vt", bufs=2)
                        nc.gpsimd.dma_start(
                            vt[:, 0, :], vdc[b * SV : b * SV + 128, rows]
                        )
                        nc.gpsimd.dma_start(
                            vt[:64, 1, :], vdc[b * SV + 128 : (b + 1) * SV, rows]
                        )
                        return vt

                    # The host permutes each half's token columns to
                    # [768 valid (4 batches x 192)] [256 masked (4 x 64)], so
                    # the k2/v GEMMs read contiguous packed slices and q/k1
                    # GEMM columns are simply the permuted token space.

                    def load_w2(w, k, cols, qi):
                        # one DMA covers two 128-row contraction chunks: HW DMA
                        # issue is ~2.2us/descriptor, so halving the count keeps
                        # the queues ahead of PE
                        wtile = wtp.tile([128, 2, 512], F32R, tag="wt", name="wtile")
                        dma_q[qi].dma_start(
                            wtile[:],
                            w.ap()[k * 128 : (k + 2) * 128, cols]
                            .bitcast(F32R)
                            .rearrange("(c p) n -> p c n", p=128),
                        )
                        return wtile

                    def phase_a(hf, hst, prefetch):
                        # q/k1: out-chunk-stationary (W tile), hsT moving.
                        # hst columns are host-permuted ([768 valid][256 masked]
                        # per half); the evac writes un-permute so qk12Td stays
                        # in original token order (single-range phase-B loads).
                        for n in ("q", "k1"):
                            wt = wts[n]
                            for mg in range(8):
                                pss = [
                                    psump.tile([128, 512], F32, tag="ps", name=f"ps{i}")
                                    for i in range(8)
                                ]
                                for k in range(0, KC, 2):
                                    wtile = load_w2(
                                        wt, k, slice(mg * 512, (mg + 1) * 512), (k // 2) % 2
                                    )
                                    for kk in range(2):
                                        for m in range(4):
                                            for nn in range(2):
                                                nc.tensor.matmul(
                                                    pss[m * 2 + nn][:],
                                                    wtile[:, kk, m * 128 : (m + 1) * 128],
                                                    hst[:, k + kk, nn * 512 : (nn + 1) * 512],
                                                    start=(k == 0 and kk == 0),
                                                    stop=(k == KC - 2 and kk == 1),
                                                )
                                for m in range(4):
                                    ev = evp.tile(
                                        [128, 2, 512], F32R, tag="ev", name="ev"
                                    )
                                    for nn in range(2):
                                        nc.scalar.activation(
                                            ev[:, nn, :],
                                            pss[m * 2 + nn][:],
                                            IDENT,
                                            bias=bias_sb[n][:, mg * 4 + m : mg * 4 + m + 1],
                                            scale=1.0,
                                        )
                                    rowsl = slice(
                                        mg * 512 + m * 128, mg * 512 + (m + 1) * 128
                                    )
                                    dst = qk12Td[
                                        proj_idx[n], rowsl, hf * HALF : (hf + 1) * HALF
                                    ].rearrange("r (b j) -> r b j", b=BPH)
                                    flat = ev[:].rearrange("p a b -> p (a b)")
                                    nc.sync.dma_start(
                                        dst[:, :, :SV],
                                        flat[:, :PHALF].rearrange(
                                            "p (b j) -> p b j", b=BPH
                                        ),
                                    )
                                    nc.sync.dma_start(
                                        dst[:, :, SV:],
                                        flat[:, PHALF:].rearrange(
                                            "p (b j) -> p b j", b=BPH
                                        ),
                                    )

                        # k2: same structure but packed tokens - each nn group
                        # covers 2 batches x 192 valid tokens = 384 columns
                        for mg in range(8):
                            pss = [
                                psump.tile([128, 512], F32, tag="ps", name=f"kps{i}")
                                for i in range(8)
                            ]
                            for k in range(0, KC, 2):
                                wtile = load_w2(
                                    wts["k2"], k, slice(mg * 512, (mg + 1) * 512),
                                    (k // 2) % 2,
                                )
                                for kk in range(2):
                                    for m in range(4):
                                        for nn in range(2):
                                            nc.tensor.matmul(
                                                pss[m * 2 + nn][:, :2 * SV],
                                                wtile[:, kk, m * 128 : (m + 1) * 128],
                                                hst[:, k + kk, nn * 2 * SV : (nn + 1) * 2 * SV],
                                                start=(k == 0 and kk == 0),
                                                stop=(k == KC - 2 and kk == 1),
                                            )
                            for m in range(4):
                                ev = evp.tile([128, 2, 512], F32R, tag="ev", name="kev")
                                for nn in range(2):
                                    nc.scalar.activation(
                                        ev[:, nn, :2 * SV],
                                        pss[m * 2 + nn][:, :2 * SV],
                                        IDENT,
                                        bias=bias_sb["k2"][:, mg * 4 + m : mg * 4 + m + 1],
                                        scale=1.0,
                                    )
                                nc.sync.dma_start(
                                    k2Td[
                                        mg * 512 + m * 128 : mg * 512 + (m + 1) * 128,
                                        hf * PHALF : (hf + 1) * PHALF,
                                    ].rearrange("r (a b) -> r a b", a=2),
                                    ev[:, :, :2 * SV],
                                )

                        # prefetch the first phase-B pairs' q/k1/k2 (ready now)
                        # so attention starts immediately after v completes
                        pre = {}
                        for b, h in prefetch:
                            pre[(b, h)] = load_qk(b, h)
                            pre[("k2", b, h)] = load_k2(b, h)

                        # v = hs@((wv1+wv2).T): single GEMM on DVE-summed tiles
                        for ng in range(8):
                            if ng == 2:
                                # vd rows for heads 0/1 (ng 0) landed during ng 1,
                                # so these loads won't block the Pool queue head
                                for b, h in prefetch[:2]:
                                    pre[("vt", b, h)] = load_vt(b, h)
                            pss = [
                                psump.tile([128, 512], F32, tag="ps", name=f"vps{i}")
                                for i in range(6)
                            ]
                            cols = slice(ng * 512, (ng + 1) * 512)
                            for k in range(0, KC, 2):
                                wv1 = load_w2(wts["v1"], k, cols, 0)
                                wv2 = load_w2(wts["v2"], k, cols, 1)
                                wsum = wtp.tile(
                                    [128, 2, 512], F32R, tag="wt", name="wsum"
                                )
                                nc.vector.tensor_add(wsum[:], wv1[:], wv2[:])
                                for kk in range(2):
                                    for m in range(6):
                                        nc.tensor.matmul(
                                            pss[m][:],
                                            hst[:, k + kk, m * 128 : (m + 1) * 128],
                                            wsum[:, kk, :],
                                            start=(k == 0 and kk == 0),
                                            stop=(k == KC - 2 and kk == 1),
                                        )
                            for mp in range(3):
                                ev = evp.tile([128, 2, 512], F32R, tag="ev", name="vev")
                                # Act engine copies: keep DVE free for wsum adds
                                # (in-order DVE queue would stall the next group)
                                for mm in range(2):
                                    nc.scalar.activation(
                                        ev[:, mm, :],
                                        pss[mp * 2 + mm][:],
                                        COPY,
                                        bias=0.0,
                                        scale=1.0,
                                    )
                                nc.sync.dma_start(
                                    vdc[
                                        hf * PHALF + mp * 256 : hf * PHALF + (mp + 1) * 256,
                                        cols,
                                    ].rearrange("(c p) n -> p c n", p=128),
                                    ev[:],
                                )
                        return pre

                    def phase_b(hf, pre):
                        for b in range(hf * BPH, (hf + 1) * BPH):
                            for h in range(H):
                                rows = slice(h * S, (h + 1) * S)
                                colsb = slice(b * S, (b + 1) * S)
                                qk = pre.pop((b, h), None)
                                if qk is None:
                                    qk = load_qk(b, h)
                                qt = qk[:, 0, :, :]
                                k1 = qk[:, 1, :, :]
                                k2 = pre.pop(("k2", b, h), None)
                                if k2 is None:
                                    k2 = load_k2(b, h)
                                vt = pre.pop(("vt", b, h), None)
                                if vt is None:
                                    vt = load_vt(b, h)

                                # s1T[m,q] = sum_d k1T[d,m] qT[d,q], scaled
                                s1r = bw.tile([128, 2, S], F32R, tag="s1r", name="s1r")
                                for m in range(2):
                                    ps = psump.tile(
                                        [128, 512], F32, tag="ps", name="s1ps"
                                    )
                                    for d_ in range(2):
                                        nc.tensor.matmul(
                                            ps[:, :S],
                                            k1[:, d_, m * 128 : (m + 1) * 128],
                                            qt[:, d_, :],
                                            start=(d_ == 0),
                                            stop=(d_ == 1),
                                        )
                                    nc.vector.tensor_scalar_mul(
                                        s1r[:, m, :], ps[:, :S], SCALE
                                    )

                                # scores[q,j] (full N=S), softmax over j<SV
                                probs = bw.tile(
                                    [128, 2, SV], F32, tag="probs", name="probs", bufs=2
                                )
                                recip = bw.tile([128, 2], F32, tag="recip", name="recip")
                                for q in range(2):
                                    ps = psump.tile(
                                        [128, 512], F32, tag="ps", name="scps"
                                    )
                                    for m in range(2):
                                        nc.tensor.matmul(
                                            ps[:, :SV],
                                            s1r[:, m, q * 128 : (q + 1) * 128],
                                            k2[:, m, :],
                                            start=(m == 0),
                                            stop=(m == 1),
                                        )
                                    negmax = bw.tile([128, 1], F32, tag="ngm", name="ngm")
                                    nc.vector.reduce_max(
                                        negmax[:], ps[:, :SV], axis=AX, negate=True
                                    )
                                    sumexp = bw.tile([128, 1], F32, tag="sme", name="sme")
                                    nc.scalar.activation(
                                        probs[:, q, :],
                                        ps[:, :SV],
                                        EXP,
                                        bias=negmax[:],
                                        scale=1.0,
                                        accum_out=sumexp[:],
                                    )
                                    nc.vector.reciprocal(recip[:, q : q + 1], sumexp[:])

                                # transpose probs (valid cols only) -> fp32r
                                ptr = bw.tile([128, 2, S], F32R, tag="ptr", name="ptr")
                                for q in range(2):
                                    pst = psump.tile(
                                        [128, 512], F32, tag="ps", name="pst"
                                    )
                                    nc.tensor.transpose(
                                        pst[:, 0:128], probs[:, q, :128], ident[:]
                                    )
                                    nc.tensor.transpose(
                                        pst[:64, 128:256], probs[:, q, 128:SV], ident[:]
                                    )
                                    # PSUM->SBUF copy on Act: DVE is B's cadence
                                    # limiter, Act has slack
                                    nc.scalar.activation(
                                        ptr[:, q, :], pst[:, :S], COPY, bias=0.0, scale=1.0
                                    )

                                # ctx[q,d] = sum_{j<SV} probsT[j,q] v[j,d];
                                # normalize (DVE) then +bias on gpsimd (idle here)
                                ctxs = bw.tile([128, 2, S], F32, tag="ctxs", name="ctxs", bufs=2)
                                for q in range(2):
                                    ps = psump.tile(
                                        [128, 512], F32, tag="ps", name="ctxps"
                                    )
                                    nc.tensor.matmul(
                                        ps[:, :S],
                                        ptr[:, q, :128],
                                        vt[:, 0, :],
                                        start=True,
                                        stop=False,
                                    )
                                    nc.tensor.matmul(
                                        ps[:, :S],
                                        ptr[:64, q, 128:256],
                                        vt[:64, 1, :],
                                        start=False,
                                        stop=True,
                                    )
                                    nc.vector.tensor_scalar_mul(
                                        ctxs[:, q, :], ps[:, :S], recip[:, q : q + 1]
                                    )
                                    nc.gpsimd.tensor_add(
                                        ctxs[:, q, :], ctxs[:, q, :], biasb[:, rows]
                                    )

                                nc.sync.dma_start(
                                    outd.ap()[colsb, rows].rearrange(
                                        "(c p) s -> p c s", p=128
                                    ),
                                    ctxs[:],
                                )

                    hst = load_hst(0)
                    for hf in range(2):
                        prefetch = [(hf * BPH, h) for h in range(2)]
                        pre = phase_a(hf, hst, prefetch)
                        if hf == 0:
                            hst = load_hst(1)  # prefetch overlaps with phase_b(0)
                        phase_b(hf, pre)

    nc.compile()
    return nc


# ---------------------------------------------------------------------------
# host-side runner (mirrors bass2jax.run_bass_via_pjrt with device-resident
# inputs; weights replicated across cores rather than concatenated)
# ---------------------------------------------------------------------------

_CACHE = {}


def _run(nc, in_maps, n_cores, replicated=(), time_reps=0):
    import jax
    from jax.sharding import Mesh, PartitionSpec, NamedSharding
    from jax.experimental.shard_map import shard_map
    from concourse.bass2jax import (
        install_neuronx_cc_hook,
        _bass_exec_p,
        partition_id_tensor,
    )

    install_neuronx_cc_hook()

    if nc.dbg_addr is not None:
        assert not nc.dbg_callbacks
        in_maps = [
            {**m, nc.dbg_addr.name: np.zeros((1, 2), np.uint32)} for m in in_maps
        ]

    partition_name = nc.partition_id_tensor.name if nc.partition_id_tensor else None

    in_names, out_names, out_avals, zero_outs = [], [], [], []
    for alloc in nc.m.functions[0].allocations:
        if not isinstance(alloc, mybir.MemoryLocationSet):
            continue
        name = alloc.memorylocations[0].name
        if alloc.kind == "ExternalInput":
            if name != partition_name:
                in_names.append(name)
        elif alloc.kind == "ExternalOutput":
            out_names.append(name)
            shape = tuple(alloc.tensor_shape)
            dtype = mybir.dt.np(alloc.dtype)
            out_avals.append(jax.core.ShapedArray(shape, dtype))
            zero_outs.append(np.zeros(shape, dtype))
    n_params = len(in_names)
    n_outs = len(out_avals)
    param_names = list(in_names)
    in_names = in_names + out_names
    if partition_name is not None:
        in_names.append(partition_name)

    donate = tuple(range(n_params, n_params + n_outs))

    def _body(*args):
        operands = list(args)
        if partition_name is not None:
            operands.append(partition_id_tensor())
        outs = _bass_exec_p.bind(
            *operands,
            out_avals=tuple(out_avals),
            in_names=tuple(in_names),
            out_names=tuple(out_names),
            lowering_input_output_aliases=(),
            sim_require_finite=True,
            sim_require_nnan=True,
            nc=nc,
        )
        return tuple(outs)

    devices = jax.devices()[:n_cores]
    mesh = Mesh(np.asarray(devices), ("core",))
    rep = set(replicated)
    in_specs = tuple(
        PartitionSpec() if nm in rep else PartitionSpec("core")
        for nm in param_names
    ) + (PartitionSpec("core"),) * n_outs
    out_specs = (PartitionSpec("core"),) * len(out_names)
    sharded = jax.jit(
        shard_map(
            _body, mesh=mesh, in_specs=in_specs, out_specs=out_specs, check_rep=False
        ),
        donate_argnums=donate,
        keep_unused=True,
    )

    shard_sh = NamedSharding(mesh, PartitionSpec("core"))
    rep_sh = NamedSharding(mesh, PartitionSpec())
    concat_in = []
    for i, nm in enumerate(param_names):
        if nm in rep:
            concat_in.append(jax.device_put(np.asarray(in_maps[0][nm]), rep_sh))
        else:
            concat_in.append(
                jax.device_put(
                    np.concatenate(
                        [np.asarray(in_maps[c][nm]) for c in range(n_cores)], axis=0
                    ),
                    shard_sh,
                )
            )
    jax.block_until_ready(concat_in)

    def fresh_zeros():
        zs = [
            jax.device_put(np.zeros((n_cores * z.shape[0], *z.shape[1:]), z.dtype), shard_sh)
            for z in zero_outs
        ]
        jax.block_until_ready(zs)
        return zs

    t0 = time.perf_counter()
    out_arrs = jax.block_until_ready(sharded(*concat_in, *fresh_zeros()))
    first_call_s = time.perf_counter() - t0
    results = [
        {
            name: np.asarray(out_arrs[i]).reshape(n_cores, *out_avals[i].shape)[c]
            for i, name in enumerate(out_names)
        }
        for c in range(n_cores)
    ]

    # non-donating variant for timing bursts: zeros stay device-resident and
    # are reused across calls (the kernel writes every output element)
    sharded_nd = jax.jit(
        shard_map(
            _body, mesh=mesh, in_specs=in_specs, out_specs=out_specs, check_rep=False
        ),
        keep_unused=True,
    )
    zs_resident = fresh_zeros()

    def timed_burst(m):
        """Enqueue m executions back-to-back, fetch a few bytes of the last
        one's output. Device serializes the execs, so wall ~= dispatch
        overhead + m * exec_time once m*exec exceeds the RPC window."""
        t0 = time.perf_counter()
        outs = None
        for _ in range(m):
            outs = sharded_nd(*concat_in, *zs_resident)
        for o in outs:
            np.asarray(jax.device_get(o.addressable_shards[0].data[0:1, 0:8]))
        return time.perf_counter() - t0

    times = [timed_burst(1) for _ in range(time_reps)]

    return results, times, first_call_s, timed_burst


def kernel(
    hidden_states,
    wq,
    bq,
    wk1,
    bk1,
    wk2,
    bk2,
    wv1,
    bv1,
    wv2,
    bv2,
    _time_reps=0,
    _reps=1,
):
    hs = np.asarray(hidden_states, dtype=np.float32)
    weights = {
        "q": np.asarray(wq, np.float32),
        "k1": np.asarray(wk1, np.float32),
        "k2": np.asarray(wk2, np.float32),
        "v1": np.asarray(wv1, np.float32),
        "v2": np.asarray(wv2, np.float32),
    }
    biases = {
        "q": np.asarray(bq, np.float32),
        "k1": np.asarray(bk1, np.float32),
        "k2": np.asarray(bk2, np.float32),
        "v1": np.asarray(bv1, np.float32),
        "v2": np.asarray(bv2, np.float32),
    }

    if ("nc", _reps) not in _CACHE:
        _CACHE[("nc", _reps)] = build_bass(_reps)
    nc = _CACHE[("nc", _reps)]

    # host prep: layout only (transpose + token-column permutation), no
    # arithmetic. Each half's tokens are reordered to [4 batches x 192
    # mask-valid][4 x 64 masked] so the k2/v GEMMs read contiguous packed
    # slices on device.
    wT = {n: np.ascontiguousarray(w.T) for n, w in weights.items()}
    perm = np.concatenate(
        [
            np.concatenate(
                [
                    np.arange(hf * HALF + bb * S, hf * HALF + bb * S + SV)
                    for bb in range(BPH)
                ]
                + [
                    np.arange(hf * HALF + bb * S + SV, hf * HALF + (bb + 1) * S)
                    for bb in range(BPH)
                ]
            )
            for hf in range(2)
        ]
    )
    in_maps = []
    for c in range(NCORES):
        shard = hs[c * BPC : (c + 1) * BPC].reshape(T, HID)[perm]
        m = {"hsT": np.ascontiguousarray(shard.T)}
        for n in ("q", "k1", "k2", "v1", "v2"):
            m[f"w{n}T"] = wT[n]
            m[f"b{n}"] = biases[n]
        in_maps.append(m)

    replicated = [f"w{n}T" for n in weights] + [f"b{n}" for n in biases]
    results, times, first_s, burst = _run(
        nc, in_maps, NCORES, replicated=replicated, time_reps=_time_reps
    )
    kernel._last_times = times
    kernel._first_call_s = first_s
    kernel._burst = burst

    out = np.empty((B, S, HID), np.float32)
    for c in range(NCORES):
        out[c * BPC : (c + 1) * BPC] = results[c]["out"].reshape(BPC, S, HID)
    return out
